# Initial kernel scaffold
#
"""Trainium2 Bass kernel for nn_MatrixModel_12884901888386.

Computes: W = where(8192 + i > j, |weight|, 0); softmax(W, axis=1)
on weight [8191, 16382] f32, sharded row-strided across 8 NeuronCores.

Sharding: core k gets global rows k, k+8, k+16, ... (1024 rows, last core
padded by one garbage row).  Row-strided sharding makes the triangular mask
boundary core-independent except for a 1024-wide diagonal band whose mask
(j_band < k + 8*p) is passed in as a tiny per-core input.

Per 128-row tile t (local rows 128t..128t+127, global row = k + 8*(128t+p)):
  cols [0, WA)        WA = 8192 + 1024t      : always kept
  cols [WA, WA+WB)    WB = min(1024, ...)    : diagonal band, mask from input
  cols [WA+WB, 16382) (width WC)             : always masked -> exp(0)=1,
                                               output = 1/rowsum broadcast
So only [0, WA+WB) is read from HBM; the all-masked tail contributes WC to
the softmax denominator and a broadcast fill to the output.
"""

import os

import numpy as np

import concourse.bacc as bacc
import concourse.tile as tile
from concourse import mybir
from concourse.bass_utils import run_bass_kernel_spmd

N_CORES = 8
ROWS_FULL = 8191
COLS = 16382
NUM_TERMS = 8192
LOCAL_ROWS = 1024  # padded so 8 * 1024 >= 8191
P = 128
N_TILES = LOCAL_ROWS // P
BAND = 1024

F32 = mybir.dt.float32
ALU = mybir.AluOpType
ACTF = mybir.ActivationFunctionType

_compiled_nc = None
last_results = None  # BassKernelResults of the most recent run (for test.py)


def _build_nc():
    nc = bacc.Bacc("TRN2", target_bir_lowering=False, debug=False,
                   num_devices=N_CORES)
    x = nc.dram_tensor("x", [LOCAL_ROWS, COLS], F32, kind="ExternalInput").ap()
    bm = nc.dram_tensor("bmask", [P, BAND], F32, kind="ExternalInput").ap()
    y = nc.dram_tensor("y", [LOCAL_ROWS, COLS], F32, kind="ExternalOutput").ap()

    with tile.TileContext(nc) as tc:
        with (
            tc.tile_pool(name="big", bufs=2) as big,
            tc.tile_pool(name="consts", bufs=1) as consts,
            tc.tile_pool(name="small", bufs=2 * N_TILES) as small,
        ):
            bmask = consts.tile([P, BAND], F32)
            nc.sync.dma_start(out=bmask, in_=bm)

            for t in range(N_TILES):
                wa = NUM_TERMS + BAND * t
                wb = min(BAND, COLS - wa)
                wab = wa + wb
                wc = COLS - wab

                xt = big.tile([P, COLS], F32, tag="xt")
                nc.sync.dma_start(out=xt[:, :wab], in_=x[t * P:(t + 1) * P, :wab])

                # |x| in place: abs_max(x, 0) on DVE (tensor_scalar, 2x f32)
                nc.vector.tensor_scalar(
                    out=xt[:, :wab], in0=xt[:, :wab],
                    scalar1=0.0, scalar2=None, op0=ALU.abs_max)

                # zero the masked part of the diagonal band
                nc.vector.tensor_tensor(
                    out=xt[:, wa:wab], in0=xt[:, wa:wab], in1=bmask[:, :wb],
                    op=ALU.mult)

                # e = exp(masked) in place, rowsum alongside (ACT engine)
                s = small.tile([P, 1], F32, tag="s")
                nc.scalar.activation(
                    out=xt[:, :wab], in_=xt[:, :wab], func=ACTF.Exp, accum_out=s)

                # denominator += WC (the all-masked tail, exp(0)=1 each)
                r = small.tile([P, 1], F32, tag="r")
                if wc > 0:
                    s2 = small.tile([P, 1], F32, tag="s2")
                    nc.vector.tensor_scalar(
                        out=s2, in0=s, scalar1=float(wc), scalar2=None,
                        op0=ALU.add)
                else:
                    s2 = s
                nc.vector.reciprocal(out=r, in_=s2)

                # out = e / rowsum (DVE tensor_scalar with per-partition scalar)
                nc.vector.tensor_scalar(
                    out=xt[:, :wab], in0=xt[:, :wab],
                    scalar1=r, scalar2=None, op0=ALU.mult)

                # all-masked tail: out = 1/rowsum broadcast (in0*0 + r)
                if wc > 0:
                    nc.vector.tensor_scalar(
                        out=xt[:, wab:], in0=xt[:, :wc],
                        scalar1=0.0, scalar2=r, op0=ALU.mult, op1=ALU.add)

                nc.sync.dma_start(out=y[t * P:(t + 1) * P, :], in_=xt)

    nc.compile()
    return nc


def _get_nc():
    global _compiled_nc
    if _compiled_nc is None:
        _compiled_nc = _build_nc()
    return _compiled_nc


def kernel(**inputs):
    global last_results
    w = np.asarray(inputs["weight"], dtype=np.float32)
    assert w.shape == (ROWS_FULL, COLS), w.shape

    in_maps = []
    for k in range(N_CORES):
        shard = w[k::N_CORES]
        if shard.shape[0] < LOCAL_ROWS:
            pad = np.zeros((LOCAL_ROWS - shard.shape[0], COLS), np.float32)
            shard = np.concatenate([shard, pad], axis=0)
        else:
            shard = np.ascontiguousarray(shard)
        p = np.arange(P)[:, None]
        j = np.arange(BAND)[None, :]
        bmask = (j < (k + N_CORES * p)).astype(np.float32)
        in_maps.append({"x": shard, "bmask": bmask})

    nc = _get_nc()
    trace = bool(os.environ.get("BASS_TRACE"))
    last_results = run_bass_kernel_spmd(
        nc, in_maps, core_ids=list(range(N_CORES)), trace=trace)

    out = np.empty((ROWS_FULL, COLS), np.float32)
    for k in range(N_CORES):
        yk = last_results.results[k]["y"]
        n_valid = len(range(k, ROWS_FULL, N_CORES))
        out[k::N_CORES] = yk[:n_valid]
    return out


# revision 3
# speedup vs baseline: 1.3953x; 1.3953x over previous
"""Trainium2 Bass kernel for nn_MatrixModel_12884901888386.

Computes: W = where(8192 + i > j, |weight|, 0); softmax(W, axis=1)
on weight [8191, 16382] f32, sharded row-strided across 8 NeuronCores.

Sharding: core k gets global rows k, k+8, k+16, ... (1024 rows, last core
padded by one garbage row).  Row-strided sharding makes the triangular mask
boundary core-independent except for a 1024-wide diagonal band whose mask
(j_band < k + 8*p) is passed in as a tiny per-core input.

Per 128-row tile t (local rows 128t..128t+127, global row = k + 8*(128t+p)):
  cols [0, WA)        WA = 8192 + 1024t      : always kept
  cols [WA, WA+WB)    WB = min(1024, ...)    : diagonal band, mask from input
  cols [WA+WB, 16382) (width WC)             : always masked -> exp(0)=1,
                                               output = 1/rowsum broadcast
So only [0, WA+WB) is read from HBM; the all-masked tail contributes WC to
the softmax denominator and a broadcast fill to the output.
"""

import os

import numpy as np

import concourse.bacc as bacc
import concourse.tile as tile
from concourse import mybir
from concourse.bass_utils import run_bass_kernel_spmd

N_CORES = 8
ROWS_FULL = 8191
COLS = 16382
NUM_TERMS = 8192
LOCAL_ROWS = 1024  # padded so 8 * 1024 >= 8191
P = 128
N_TILES = LOCAL_ROWS // P
BAND = 1024

F32 = mybir.dt.float32
ALU = mybir.AluOpType
ACTF = mybir.ActivationFunctionType

_compiled_nc = None
last_results = None  # BassKernelResults of the most recent run (for test.py)


def _build_nc(n_reps=1):
    nc = bacc.Bacc("TRN2", target_bir_lowering=False, debug=False,
                   num_devices=N_CORES)
    x = nc.dram_tensor("x", [LOCAL_ROWS, COLS], F32, kind="ExternalInput").ap()
    bm = nc.dram_tensor("bmask", [P, BAND], F32, kind="ExternalInput").ap()
    y = nc.dram_tensor("y", [LOCAL_ROWS, COLS], F32, kind="ExternalOutput").ap()

    with tile.TileContext(nc) as tc:
        with (
            tc.tile_pool(name="big", bufs=2) as big,
            tc.tile_pool(name="consts", bufs=1) as consts,
            tc.tile_pool(name="small", bufs=2 * N_TILES) as small,
        ):
            bmask = consts.tile([P, BAND], F32)
            nc.sync.dma_start(out=bmask, in_=bm)

            for t in range(N_TILES * n_reps):
                t = t % N_TILES
                wa = NUM_TERMS + BAND * t
                wb = min(BAND, COLS - wa)
                wab = wa + wb
                wc = COLS - wab

                xt = big.tile([P, COLS], F32, tag="xt")
                nc.sync.dma_start(out=xt[:, :wab], in_=x[t * P:(t + 1) * P, :wab])

                # |x| in place on ACT (Abs is a filler function in every table set)
                nc.scalar.activation(
                    out=xt[:, :wab], in_=xt[:, :wab], func=ACTF.Abs)

                # zero the masked part of the diagonal band
                nc.vector.tensor_tensor(
                    out=xt[:, wa:wab], in0=xt[:, wa:wab], in1=bmask[:, :wb],
                    op=ALU.mult)

                # e = exp(masked) in place, rowsum alongside (ACT engine)
                s = small.tile([P, 1], F32, tag="s")
                nc.scalar.activation(
                    out=xt[:, :wab], in_=xt[:, :wab], func=ACTF.Exp, accum_out=s)

                # denominator += WC (the all-masked tail, exp(0)=1 each)
                r = small.tile([P, 1], F32, tag="r")
                if wc > 0:
                    s2 = small.tile([P, 1], F32, tag="s2")
                    nc.vector.tensor_scalar(
                        out=s2, in0=s, scalar1=float(wc), scalar2=None,
                        op0=ALU.add)
                else:
                    s2 = s
                nc.vector.reciprocal(out=r, in_=s2)

                # out = e / rowsum (DVE tensor_scalar with per-partition scalar)
                nc.vector.tensor_scalar(
                    out=xt[:, :wab], in0=xt[:, :wab],
                    scalar1=r, scalar2=None, op0=ALU.mult)

                # all-masked tail: out = 1/rowsum broadcast (in0*0 + r)
                if wc > 0:
                    nc.vector.tensor_scalar(
                        out=xt[:, wab:], in0=xt[:, :wc],
                        scalar1=0.0, scalar2=r, op0=ALU.mult, op1=ALU.add)

                nc.sync.dma_start(out=y[t * P:(t + 1) * P, :], in_=xt)

    nc.compile()
    return nc


def _get_nc():
    global _compiled_nc
    if _compiled_nc is None:
        _compiled_nc = _build_nc()
    return _compiled_nc


def kernel(**inputs):
    global last_results
    w = np.asarray(inputs["weight"], dtype=np.float32)
    assert w.shape == (ROWS_FULL, COLS), w.shape

    in_maps = []
    for k in range(N_CORES):
        shard = w[k::N_CORES]
        if shard.shape[0] < LOCAL_ROWS:
            pad = np.zeros((LOCAL_ROWS - shard.shape[0], COLS), np.float32)
            shard = np.concatenate([shard, pad], axis=0)
        else:
            shard = np.ascontiguousarray(shard)
        p = np.arange(P)[:, None]
        j = np.arange(BAND)[None, :]
        bmask = (j < (k + N_CORES * p)).astype(np.float32)
        in_maps.append({"x": shard, "bmask": bmask})

    nc = _get_nc()
    trace = bool(os.environ.get("BASS_TRACE"))
    last_results = run_bass_kernel_spmd(
        nc, in_maps, core_ids=list(range(N_CORES)), trace=trace)

    out = np.empty((ROWS_FULL, COLS), np.float32)
    for k in range(N_CORES):
        yk = last_results.results[k]["y"]
        n_valid = len(range(k, ROWS_FULL, N_CORES))
        out[k::N_CORES] = yk[:n_valid]
    return out
